# revision 4
# baseline (speedup 1.0000x reference)
"""Trainium2 Bass kernel for an 8-batch BERT block (nn_BERTBlock_13958643712031).

Sharding: pure data-parallel over batch (B=8 == n_cores). Each NeuronCore
computes the full transformer block for one batch element; no collectives.

Per-core dataflow (S=1024, E=1024, H=16 heads, DH=64, HID=4096):
  - QKV projections produce qT/kT [head*DH, S] and v [S, head*DH] (bf16).
  - Attention per head works in "scoresT" layout [s_key, s_query] so the
    softmax sum reduces over the PSUM partition axis via the matmul itself:
    v is augmented with a ones-column, so o^T = [v|1]^T @ p yields both the
    unnormalized context rows and the softmax denominator row in one pass.
  - Softmax skips the max-subtraction (scores are O(1); exp is exact in fp32
    modulo rounding) which matches the reference within fp32 noise.
  - Residual stream (h2, a, h3) kept in fp32; matmul operands in bf16.
  - g1/beta1 are folded into w1/b1 on the host (exact fp32 math).
"""

import os
import sys

import numpy as np
import ml_dtypes

sys.path.insert(0, "/opt/trn_rl_repo")

B, S, E, H, DH, HID = 8, 1024, 1024, 16, 64, 4096
P = 128
NT = S // P     # 8 sequence tiles
KE = E // P     # 8 embedding k-tiles
HT = HID // P   # 32 hidden tiles
EPS_LN = 1e-5

BF16 = ml_dtypes.bfloat16

_PROGRAM_CACHE = {}


def _build_program(apply_mask: bool, sim_safe_gelu: bool = False):
    import concourse.bass as bass
    import concourse.tile as tile
    from concourse import bacc, mybir
    from concourse.masks import make_identity

    bf = mybir.dt.bfloat16
    f32 = mybir.dt.float32
    AF = mybir.ActivationFunctionType
    ALU = mybir.AluOpType

    nc = bacc.Bacc("TRN2", target_bir_lowering=False, debug=False)

    # ---- DRAM I/O ----
    hT_d = nc.dram_tensor("hT", [E, S], bf, kind="ExternalInput")
    h_d = nc.dram_tensor("h", [S, E], f32, kind="ExternalInput")
    wqkvT_d = nc.dram_tensor("wqkvT", [E, 3 * E], bf, kind="ExternalInput")
    wmhT_d = nc.dram_tensor("wmhT", [E, E], bf, kind="ExternalInput")
    w1T_d = nc.dram_tensor("w1T", [E, HID], bf, kind="ExternalInput")
    b1c_d = nc.dram_tensor("b1c", [P, HT], f32, kind="ExternalInput")
    w2T_d = nc.dram_tensor("w2T", [HID, E], bf, kind="ExternalInput")
    b2r_d = nc.dram_tensor("b2r", [1, E], f32, kind="ExternalInput")
    g2r_d = nc.dram_tensor("g2r", [1, E], f32, kind="ExternalInput")
    beta2r_d = nc.dram_tensor("beta2r", [1, E], f32, kind="ExternalInput")
    mcol_d = nc.dram_tensor("mcol", [P, NT], f32, kind="ExternalInput")
    if apply_mask:
        maskT_d = nc.dram_tensor("maskT", [S, S], f32, kind="ExternalInput")
    out_d = nc.dram_tensor("out", [S, E], f32, kind="ExternalOutput")

    gelu_func = AF.Tanh if sim_safe_gelu else AF.Gelu

    with tile.TileContext(nc) as tc:
        # ---------- constants ----------
        const = tc.alloc_tile_pool(name="const", bufs=1)
        ident = const.tile([P, P], bf)
        make_identity(nc, ident)
        eps_t = const.tile([P, 1], f32)
        nc.vector.memset(eps_t, EPS_LN)
        b1_sb = const.tile([P, HT], f32)
        nc.sync.dma_start(out=b1_sb, in_=b1c_d[:, :])
        mcol_sb = const.tile([P, NT], f32)
        nc.sync.dma_start(out=mcol_sb, in_=mcol_d[:, :])
        b2b = const.tile([P, E], f32)
        g2b = const.tile([P, E], f32)
        beta2b = const.tile([P, E], f32)
        with tc.tile_pool(name="rows_tmp", bufs=1) as rows_tmp:
            rows_sb = rows_tmp.tile([1, 3 * E], f32)
            nc.sync.dma_start(out=rows_sb[0:1, 0:E], in_=b2r_d[:, :])
            nc.sync.dma_start(out=rows_sb[0:1, E:2 * E], in_=g2r_d[:, :])
            nc.sync.dma_start(out=rows_sb[0:1, 2 * E:3 * E], in_=beta2r_d[:, :])
            nc.gpsimd.partition_broadcast(out_ap=b2b, in_ap=rows_sb[0:1, 0:E])
            nc.gpsimd.partition_broadcast(out_ap=g2b, in_ap=rows_sb[0:1, E:2 * E])
            nc.gpsimd.partition_broadcast(out_ap=beta2b,
                                          in_ap=rows_sb[0:1, 2 * E:3 * E])

        # persistent activations
        persist = tc.alloc_tile_pool(name="persist", bufs=1)
        oT_sb = persist.tile([P, KE, S], bf)       # [head*DH, S] stacked o^T
        a_sb = persist.tile([P, NT, E], f32)       # post-attn layernorm (fp32)
        aT_sb = persist.tile([P, KE, S], bf)       # a transposed, bf16

        # ---------- phase A: QKV + attention ----------
        with tc.tile_pool(name="attn_big", bufs=1) as abig, \
             tc.tile_pool(name="sc_ps", bufs=2, space="PSUM") as sc_psp, \
             tc.tile_pool(name="o_ps", bufs=3, space="PSUM") as o_psp:

            qT_sb = abig.tile([P, KE, S], bf)
            kT_sb = abig.tile([P, KE, S], bf)
            # v augmented with a ones column: [p, sk_tile, head, 65]
            v_sb = abig.tile([P, NT, H, DH + 1], bf)
            for i in range(NT):
                nc.gpsimd.memset(v_sb[:, i, :, DH], 1.0)

            if apply_mask:
                maskT_sb = abig.tile([P, NT, S], f32)
                for i in range(NT):
                    nc.sync.dma_start(out=maskT_sb[:, i, :],
                                      in_=maskT_d[i * P:(i + 1) * P, :])

            with tc.tile_pool(name="qkv_in", bufs=1) as qkvin, \
                 tc.tile_pool(name="qkv_ps", bufs=2, space="PSUM") as qkv_ps:
                hT_sb = qkvin.tile([P, KE, S], bf)
                for k in range(KE):
                    nc.sync.dma_start(out=hT_sb[:, k, :],
                                      in_=hT_d[k * P:(k + 1) * P, :])
                wqkv_sb = []
                for k in range(KE):
                    wt = qkvin.tile([P, 3 * E], bf, name=f"wqkv_{k}")
                    nc.sync.dma_start(out=wt, in_=wqkvT_d[k * P:(k + 1) * P, :])
                    wqkv_sb.append(wt)

                # q/k projections: out rows are (head, dh); columns are tokens
                for m in range(2 * KE):
                    dst = qT_sb if m < KE else kT_sb
                    j = m % KE
                    for half in range(2):
                        ps = qkv_ps.tile([P, 512], f32, tag="qkvps",
                                         name=f"qkps_{m}_{half}")
                        for k in range(KE):
                            nc.tensor.matmul(
                                ps,
                                lhsT=wqkv_sb[k][:, m * P:(m + 1) * P],
                                rhs=hT_sb[:, k, half * 512:(half + 1) * 512],
                                start=(k == 0), stop=(k == KE - 1),
                            )
                        nc.vector.tensor_copy(
                            dst[:, j, half * 512:(half + 1) * 512], ps)
                # v projection: natural layout [token, head*dh]
                for ms in range(NT):
                    for vh in range(2):  # half of the head*dh axis
                        ps = qkv_ps.tile([P, 512], f32, tag="qkvps",
                                         name=f"vps_{ms}_{vh}")
                        for k in range(KE):
                            nc.tensor.matmul(
                                ps,
                                lhsT=hT_sb[:, k, ms * P:(ms + 1) * P],
                                rhs=wqkv_sb[k][:, 2 * E + vh * 512:
                                               2 * E + (vh + 1) * 512],
                                start=(k == 0), stop=(k == KE - 1),
                            )
                        # scatter 8 heads' [P, 64] into the augmented v layout
                        nc.vector.tensor_copy(
                            v_sb[:, ms, vh * 8:(vh + 1) * 8, 0:DH],
                            ps.rearrange("p (h d) -> p h d", d=DH),
                        )

            with tc.tile_pool(name="p_pool", bufs=2) as p_pool, \
                 tc.tile_pool(name="attn_small", bufs=2) as asmall:
                # attention, head by head
                for h in range(H):
                    j, r = h // 2, (h % 2) * 64
                    pT = p_pool.tile([P, NT, S], bf, tag="pT", name=f"pT_{h}")
                    o_ps = [
                        o_psp.tile([P, 512], f32, tag="ops", name=f"ops_{h}_{hf}")
                        for hf in range(2)
                    ]
                    for i in range(NT):
                        for half in range(2):
                            sq = slice(half * 512, (half + 1) * 512)
                            sc = sc_psp.tile([P, 512], f32, tag="scps",
                                             name=f"sc_{h}_{i}_{half}")
                            nc.tensor.matmul(
                                sc,
                                lhsT=kT_sb[r:r + 64, j, i * P:(i + 1) * P],
                                rhs=qT_sb[r:r + 64, j, sq],
                                start=True, stop=True,
                            )
                            if apply_mask:
                                nc.vector.tensor_mul(sc, sc, maskT_sb[:, i, sq])
                            nc.scalar.activation(out=pT[:, i, sq], in_=sc,
                                                 func=AF.Exp, scale=0.125)
                            if apply_mask:
                                nc.vector.tensor_mul(pT[:, i, sq], pT[:, i, sq],
                                                     maskT_sb[:, i, sq])
                    for i in range(NT):
                        for half in range(2):
                            sq = slice(half * 512, (half + 1) * 512)
                            nc.tensor.matmul(
                                o_ps[half][0:DH + 1, :],
                                lhsT=v_sb[:, i, h, :],
                                rhs=pT[:, i, sq],
                                start=(i == 0), stop=(i == NT - 1),
                            )
                    for half in range(2):
                        sq = slice(half * 512, (half + 1) * 512)
                        rec = asmall.tile([P, 512], f32, tag="rec",
                                          name=f"rec_{h}_{half}")
                        if apply_mask:
                            nc.vector.tensor_scalar_add(
                                o_ps[half][DH:DH + 1, :],
                                o_ps[half][DH:DH + 1, :], 1e-20)
                        nc.vector.reciprocal(out=rec[0:1, :],
                                             in_=o_ps[half][DH:DH + 1, :])
                        bc = asmall.tile([64, 512], f32, tag="bc",
                                         name=f"bc_{h}_{half}")
                        nc.gpsimd.partition_broadcast(out_ap=bc, in_ap=rec[0:1, :])
                        nc.vector.tensor_mul(
                            oT_sb[r:r + 64, j, sq], o_ps[half][0:DH, :], bc)

        # ---------- phase B: mh + residual + layernorm1 + transpose ----------
        with tc.tile_pool(name="mh_w", bufs=1) as mhw_pool, \
             tc.tile_pool(name="resid", bufs=2) as resid, \
             tc.tile_pool(name="stat", bufs=4) as statp, \
             tc.tile_pool(name="mh_ps", bufs=2, space="PSUM") as mh_psp, \
             tc.tile_pool(name="tr_ps", bufs=2, space="PSUM") as tr_psp:

            wmh_sb = mhw_pool.tile([P, KE, E], bf)
            for k in range(KE):
                nc.sync.dma_start(out=wmh_sb[:, k, :],
                                  in_=wmhT_d[k * P:(k + 1) * P, :])

            for t in range(NT):
                h_t = resid.tile([P, E], f32, tag="h_t", name=f"h_{t}")
                nc.sync.dma_start(out=h_t, in_=h_d[t * P:(t + 1) * P, :])
                h2 = resid.tile([P, E], f32, tag="h2", name=f"h2_{t}")
                for half in range(2):
                    se = slice(half * 512, (half + 1) * 512)
                    ps = mh_psp.tile([P, 512], f32, tag="mhps",
                                     name=f"mhps_{t}_{half}")
                    for k in range(KE):
                        nc.tensor.matmul(
                            ps,
                            lhsT=oT_sb[:, k, t * P:(t + 1) * P],
                            rhs=wmh_sb[:, k, se],
                            start=(k == 0), stop=(k == KE - 1),
                        )
                    nc.vector.tensor_add(h2[:, se], h_t[:, se], ps)
                st = statp.tile([P, 2, 6], f32, tag="st", name=f"st_{t}")
                nc.vector.bn_stats(out=st[:, 0, :], in_=h2[:, 0:512])
                nc.vector.bn_stats(out=st[:, 1, :], in_=h2[:, 512:1024])
                mv = statp.tile([P, 2], f32, tag="mv", name=f"mv_{t}")
                nc.vector.bn_aggr(out=mv, in_=st)
                std = statp.tile([P, 1], f32, tag="std", name=f"std_{t}")
                nc.scalar.activation(out=std, in_=mv[:, 1:2], func=AF.Sqrt,
                                     bias=eps_t, scale=1.0)
                rstd = statp.tile([P, 1], f32, tag="rstd", name=f"rstd_{t}")
                nc.vector.reciprocal(out=rstd, in_=std)
                nc.vector.tensor_scalar(
                    out=a_sb[:, t, :], in0=h2, scalar1=mv[:, 0:1], scalar2=rstd,
                    op0=ALU.subtract, op1=ALU.mult)
                a_bf = resid.tile([P, E], bf, tag="a_bf", name=f"abf_{t}")
                nc.gpsimd.tensor_copy(out=a_bf, in_=a_sb[:, t, :])
                for jj in range(KE):
                    trp = tr_psp.tile([P, P], bf, tag="trps",
                                      name=f"tr_{t}_{jj}")
                    nc.tensor.transpose(trp, a_bf[:, jj * P:(jj + 1) * P], ident)
                    nc.vector.tensor_copy(aT_sb[:, jj, t * P:(t + 1) * P], trp)

        # ---------- phase C: FFN + residual + layernorm2 ----------
        with tc.tile_pool(name="w1_pool", bufs=1) as w1_pool, \
             tc.tile_pool(name="w2_pool", bufs=3) as w2_pool, \
             tc.tile_pool(name="g_pool", bufs=1) as g_pool, \
             tc.tile_pool(name="ffn_tmp", bufs=1) as ftmp, \
             tc.tile_pool(name="stat2", bufs=4) as statp2:

            w1_sb = []
            for k in range(KE):
                wt = w1_pool.tile([P, HID], bf, name=f"w1_{k}")
                nc.sync.dma_start(out=wt, in_=w1T_d[k * P:(k + 1) * P, :])
                w1_sb.append(wt)

            for sqh in range(2):  # sequence halves of 512 tokens
                sq = slice(sqh * 512, (sqh + 1) * 512)
                g_sb = g_pool.tile([P, HT, 512], bf, tag="g", name=f"g_{sqh}")
                with tc.tile_pool(name=f"f1_ps{sqh}", bufs=2,
                                  space="PSUM") as f1_psp:
                    for m in range(HT):
                        ps = f1_psp.tile([P, 512], f32, tag="f1ps",
                                         name=f"f1ps_{sqh}_{m}")
                        for k in range(KE):
                            nc.tensor.matmul(
                                ps,
                                lhsT=w1_sb[k][:, m * P:(m + 1) * P],
                                rhs=aT_sb[:, k, sq],
                                start=(k == 0), stop=(k == KE - 1),
                            )
                        nc.scalar.activation(out=g_sb[:, m, :], in_=ps,
                                             func=gelu_func,
                                             bias=b1_sb[:, m:m + 1], scale=1.0)
                # f2: 8 psum banks cover (4 seq tiles) x (2 E halves)
                with tc.tile_pool(name=f"f2_ps{sqh}", bufs=8,
                                  space="PSUM") as f2_psp:
                    f2_ps = [[f2_psp.tile([P, 512], f32, tag="f2ps",
                                          name=f"f2ps_{sqh}_{t2}_{eh}")
                              for eh in range(2)] for t2 in range(4)]
                    for k2 in range(HT):
                        w2_t = w2_pool.tile([P, E], bf, tag="w2",
                                            name=f"w2_{sqh}_{k2}")
                        nc.sync.dma_start(out=w2_t,
                                          in_=w2T_d[k2 * P:(k2 + 1) * P, :])
                        for t2 in range(4):
                            for eh in range(2):
                                nc.tensor.matmul(
                                    f2_ps[t2][eh],
                                    lhsT=g_sb[:, k2, t2 * P:(t2 + 1) * P],
                                    rhs=w2_t[:, eh * 512:(eh + 1) * 512],
                                    start=(k2 == 0), stop=(k2 == HT - 1),
                                )
                    for t2 in range(4):
                        t = sqh * 4 + t2
                        h3 = ftmp.tile([P, E], f32, tag="big", bufs=3,
                                       name=f"h3_{t}")
                        for eh in range(2):
                            se = slice(eh * 512, (eh + 1) * 512)
                            fb = ftmp.tile([P, 512], f32, tag="fb", bufs=2,
                                           name=f"fb_{t}_{eh}")
                            nc.vector.tensor_add(fb, f2_ps[t2][eh], b2b[:, se])
                            nc.vector.tensor_scalar_mul(fb, fb,
                                                        mcol_sb[:, t:t + 1])
                            nc.vector.tensor_add(h3[:, se], a_sb[:, t, se], fb)
                        st2 = statp2.tile([P, 2, 6], f32, tag="st2",
                                          name=f"st2_{t}")
                        nc.vector.bn_stats(out=st2[:, 0, :], in_=h3[:, 0:512])
                        nc.vector.bn_stats(out=st2[:, 1, :], in_=h3[:, 512:1024])
                        mv2 = statp2.tile([P, 2], f32, tag="mv2", name=f"mv2_{t}")
                        nc.vector.bn_aggr(out=mv2, in_=st2)
                        std2 = statp2.tile([P, 1], f32, tag="std2",
                                           name=f"std2_{t}")
                        nc.scalar.activation(out=std2, in_=mv2[:, 1:2],
                                             func=AF.Sqrt, bias=eps_t, scale=1.0)
                        rstd2 = statp2.tile([P, 1], f32, tag="rstd2",
                                            name=f"rstd2_{t}")
                        nc.vector.reciprocal(out=rstd2, in_=std2)
                        xo = ftmp.tile([P, E], f32, tag="big", bufs=3,
                                       name=f"xo_{t}")
                        nc.vector.tensor_scalar(
                            out=xo, in0=h3, scalar1=mv2[:, 0:1], scalar2=rstd2,
                            op0=ALU.subtract, op1=ALU.mult)
                        nc.vector.tensor_mul(xo, xo, g2b)
                        out_t = ftmp.tile([P, E], f32, tag="big", bufs=3,
                                          name=f"out_{t}")
                        nc.vector.tensor_add(out_t, xo, beta2b)
                        nc.sync.dma_start(out=out_d[t * P:(t + 1) * P, :],
                                          in_=out_t)

        persist.release()
        const.release()

    nc.compile()
    return nc


def _prep_inputs(h, mask, wq, wk, wv, w_mh, g1, beta1, w1, b1, w2, b2, g2, beta2):
    """Host-side packing. Returns (in_maps, apply_mask)."""
    f32 = np.float32
    h = np.asarray(h, f32)
    mask = np.asarray(mask, f32)
    apply_mask = not bool(np.all(mask == 1.0))

    wq2 = np.asarray(wq, f32).reshape(H * DH, E)
    wk2 = np.asarray(wk, f32).reshape(H * DH, E)
    wv2 = np.asarray(wv, f32).reshape(H * DH, E)
    wqkvT = np.ascontiguousarray(
        np.concatenate([wq2, wk2, wv2], axis=0).T).astype(BF16)
    wmhT = np.ascontiguousarray(np.asarray(w_mh, f32).T).astype(BF16)

    g1 = np.asarray(g1, f32)
    beta1 = np.asarray(beta1, f32)
    w1 = np.asarray(w1, f32)
    b1 = np.asarray(b1, f32)
    b1f = b1 + w1 @ beta1
    w1T = np.ascontiguousarray((w1 * g1[None, :]).T).astype(BF16)
    b1c = np.ascontiguousarray(b1f.reshape(HT, P).T).astype(f32)
    w2T = np.ascontiguousarray(np.asarray(w2, f32).T).astype(BF16)
    b2r = np.asarray(b2, f32).reshape(1, E)
    g2r = np.asarray(g2, f32).reshape(1, E)
    beta2r = np.asarray(beta2, f32).reshape(1, E)

    shared = {
        "wqkvT": wqkvT, "wmhT": wmhT, "w1T": w1T, "b1c": b1c,
        "w2T": w2T, "b2r": b2r, "g2r": g2r, "beta2r": beta2r,
    }
    in_maps = []
    for c in range(B):
        m = dict(shared)
        m["hT"] = np.ascontiguousarray(h[c].T).astype(BF16)
        m["h"] = np.ascontiguousarray(h[c])
        m["mcol"] = np.ascontiguousarray(
            mask[c][:, -1].reshape(NT, P).T).astype(f32)
        if apply_mask:
            m["maskT"] = np.ascontiguousarray(mask[c].T).astype(f32)
        in_maps.append(m)
    return in_maps, apply_mask


def kernel(**inputs) -> np.ndarray:
    from concourse.bass_utils import run_bass_kernel_spmd

    in_maps, apply_mask = _prep_inputs(**inputs)
    key = (apply_mask,)
    if key not in _PROGRAM_CACHE:
        _PROGRAM_CACHE[key] = _build_program(apply_mask)
    nc = _PROGRAM_CACHE[key]

    res = run_bass_kernel_spmd(nc, in_maps, core_ids=list(range(B)))
    out = np.stack([np.asarray(r["out"], np.float32) for r in res.results])
    return out


if __name__ == "__main__":
    import reference as R

    inputs = {k: np.asarray(v) for k, v in R.setup_inputs().items()}
    out = kernel(**inputs)
    print("out", out.shape, out.dtype)
